# revision 5
# baseline (speedup 1.0000x reference)
"""Trainium2 Bass kernel for nn_Block_65661460021475.

Computation (per reference, batch=1):
    x   = w1 @ relu(cp)                       # (128, F)
    y   = fft_convolve(x, dec_curve)          # == IIR: y[t] = dec*(y[t-1]+x[t])
    x2  = w2 @ y + x
    cp_out = tanh(g * x2)
    audio_out[w, f] = sum_c audio[w, c] cp_out[c, f]   -> stored (f, w)

Key reformulations:
  * The decay curve dec^(k+1) (dec in (0.75, 0.87)) underflows to zero in
    f32 after ~700 frames, so the length-131072 FFT convolution is exactly
    a first-order IIR filter.  It runs natively on the VectorEngine via
    tensor_tensor_scan: state = (x[t] + state) * dec.
  * Frames are sharded over 8 cores with a 512-frame halo warm-up for the
    scan (truncation error ~dec^512 ~ 1e-33, far below f32 eps).
  * The residual "+ x" is folded algebraically: x[t] = y[t]/dec - y[t-1],
    so  x2 = (w2 + diag(1/dec)) @ y - shift(y).  The shift term is applied
    as a -I matmul accumulated into the same PSUM bank, which avoids ever
    materializing x in SBUF.
"""

from contextlib import ExitStack

import numpy as np

import concourse.bacc as bacc
import concourse.bass as bass  # noqa: F401  (AP helpers)
import concourse.mybir as mybir
import concourse.tile as tile
from concourse.bass_utils import run_bass_kernel_spmd

N_CORES = 8
D = 128              # channels (= partitions)
FRAMES = 65536
WINDOW = 512
SHARD = FRAMES // N_CORES      # 8192 frames per core
HALO = 512                     # IIR warm-up halo
CHUNK = SHARD + HALO           # 8704 frames processed per core
FT = 512                       # frame tile
NT = CHUNK // FT               # 17 tiles (tile 0 = halo, no outputs)

F32 = mybir.dt.float32

_CACHE = {}

# Results of the most recent device run (for test harness introspection).
LAST_RESULTS = None


def _emit(nc):
    cp_h = nc.dram_tensor("cp", [D, CHUNK], F32, kind="ExternalInput")
    w1t_h = nc.dram_tensor("w1t", [D, D], F32, kind="ExternalInput")
    w2pt_h = nc.dram_tensor("w2pt", [D, D], F32, kind="ExternalInput")
    nid_h = nc.dram_tensor("negident", [D, D], F32, kind="ExternalInput")
    audiot_h = nc.dram_tensor("audiot", [D, WINDOW], F32, kind="ExternalInput")
    decb_h = nc.dram_tensor("decb", [D, FT], F32, kind="ExternalInput")
    g_h = nc.dram_tensor("g", [D, 1], F32, kind="ExternalInput")

    cpout_h = nc.dram_tensor("cp_out", [D, SHARD], F32, kind="ExternalOutput")
    audio_h = nc.dram_tensor("audio_out", [SHARD, WINDOW], F32, kind="ExternalOutput")

    # DRAM view of audio_out ordered (tile, p, q, w): one 1 MiB DMA per
    # frame tile stores the SBUF (p=128, q=4, w=512) staging tile; frame
    # index = tile*512 + q*128 + p.
    audio_view = audio_h[:, :].rearrange("(t q p) w -> t p q w", q=4, p=D)

    with ExitStack() as ctx:
        tc = ctx.enter_context(tile.TileContext(nc))
        consts = ctx.enter_context(tc.tile_pool(name="consts", bufs=1))
        cpin = ctx.enter_context(tc.tile_pool(name="cpin", bufs=4))
        relup = ctx.enter_context(tc.tile_pool(name="relup", bufs=3))
        ybig = ctx.enter_context(tc.tile_pool(name="ybig", bufs=1))
        cpoutp = ctx.enter_context(tc.tile_pool(name="cpoutp", bufs=3))
        aup = ctx.enter_context(tc.tile_pool(name="aup", bufs=3))
        psx = ctx.enter_context(tc.tile_pool(name="psx", bufs=2, space="PSUM"))
        psb = ctx.enter_context(tc.tile_pool(name="psb", bufs=2, space="PSUM"))
        psa = ctx.enter_context(tc.tile_pool(name="psa", bufs=4, space="PSUM"))

        w1t_sb = consts.tile([D, D], F32)
        w2pt_sb = consts.tile([D, D], F32)
        nid_sb = consts.tile([D, D], F32)
        audiot_sb = consts.tile([D, WINDOW], F32)
        decb_sb = consts.tile([D, FT], F32)
        g_sb = consts.tile([D, 1], F32)
        nc.sync.dma_start(out=w1t_sb, in_=w1t_h[:, :])
        nc.sync.dma_start(out=w2pt_sb, in_=w2pt_h[:, :])
        nc.sync.dma_start(out=nid_sb, in_=nid_h[:, :])
        nc.sync.dma_start(out=audiot_sb, in_=audiot_h[:, :])
        nc.sync.dma_start(out=decb_sb, in_=decb_h[:, :])
        nc.sync.dma_start(out=g_sb, in_=g_h[:, :])

        # y for the whole chunk lives in one contiguous SBUF buffer so the
        # scan carry and the shifted view are plain slices (34 KiB/partition).
        y_big = ybig.tile([D, CHUNK], F32)

        for t in range(NT):
            f0 = t * FT
            cp_t = cpin.tile([D, FT], F32, tag="cp_t")
            nc.sync.dma_start(out=cp_t, in_=cp_h[:, f0:f0 + FT])

            relu_t = relup.tile([D, FT], F32, tag="relu_t")
            nc.gpsimd.tensor_scalar_max(relu_t, cp_t, 0.0)

            px = psx.tile([D, FT], F32, tag="px")
            nc.tensor.matmul(px, w1t_sb, relu_t, start=True, stop=True)

            y_t = y_big[:, f0:f0 + FT]
            init = 0.0 if t == 0 else y_big[:, f0 - 1:f0]
            nc.vector.tensor_tensor_scan(
                y_t, px, decb_sb, init,
                op0=mybir.AluOpType.add, op1=mybir.AluOpType.mult,
            )

            if t == 0:
                continue
            ot = t - 1  # output tile index

            # x2 = (w2 + diag(1/dec)) @ y - shift(y)   (== w2 @ y + x)
            pb = psb.tile([D, FT], F32, tag="pb")
            nc.tensor.matmul(pb, w2pt_sb, y_t, start=True, stop=False)
            nc.tensor.matmul(pb, nid_sb, y_big[:, f0 - 1:f0 + FT - 1],
                             start=False, stop=True)

            co_t = cpoutp.tile([D, FT], F32, tag="co_t")
            nc.scalar.activation(
                co_t, pb, mybir.ActivationFunctionType.Tanh,
                scale=g_sb[:, 0:1],
            )
            nc.sync.dma_start(out=cpout_h[:, ot * FT:(ot + 1) * FT], in_=co_t)

            au_sb = aup.tile([D, 4, WINDOW], F32, tag="au_sb")
            for q in range(4):
                pa = psa.tile([D, WINDOW], F32, tag="pa")
                nc.tensor.matmul(
                    pa, co_t[:, q * 128:(q + 1) * 128], audiot_sb,
                    start=True, stop=True,
                )
                if q % 2 == 0:
                    nc.vector.tensor_copy(au_sb[:, q, :], pa)
                else:
                    nc.scalar.copy(au_sb[:, q, :], pa)
            nc.sync.dma_start(out=audio_view[ot], in_=au_sb)

    return nc


def _get_nc():
    if "nc" not in _CACHE:
        nc = _emit(bacc.Bacc())
        nc.compile()  # bacc passes: split multi-waits, reg alloc, nop fusion
        _CACHE["nc"] = nc
    return _CACHE["nc"]


def _host_prep(cp, w1, w2, audio, decays, gains):
    cp = np.asarray(cp, dtype=np.float32)
    w1 = np.asarray(w1, dtype=np.float32)
    w2 = np.asarray(w2, dtype=np.float32)
    audio = np.asarray(audio, dtype=np.float32)
    decays64 = np.asarray(decays, dtype=np.float64)
    gains64 = np.asarray(gains, dtype=np.float64)

    dec64 = 1e-12 + 0.5 + (1.0 / (1.0 + np.exp(-decays64))) * 0.5
    dec = dec64.astype(np.float32)
    g = ((1.0 / (1.0 + np.exp(-gains64))) * 5.0).astype(np.float32)

    w1t = np.ascontiguousarray(w1.T)
    w2pt = np.ascontiguousarray(w2.T + np.diag(1.0 / dec64)).astype(np.float32)
    negident = (-np.eye(D)).astype(np.float32)
    audiot = np.ascontiguousarray(audio.T)
    decb = np.ascontiguousarray(np.broadcast_to(dec[:, None], (D, FT)))
    g2 = np.ascontiguousarray(g[:, None])

    cp_pad = np.concatenate([np.zeros((D, HALO), np.float32), cp[0]], axis=1)
    in_maps = []
    for k in range(N_CORES):
        sl = np.ascontiguousarray(cp_pad[:, k * SHARD:k * SHARD + CHUNK])
        in_maps.append({
            "cp": sl, "w1t": w1t, "w2pt": w2pt, "negident": negident,
            "audiot": audiot, "decb": decb, "g": g2,
        })
    return in_maps


def kernel(cp, w1, w2, audio, decays, gains):
    global LAST_RESULTS
    in_maps = _host_prep(cp, w1, w2, audio, decays, gains)
    nc = _get_nc()
    res = run_bass_kernel_spmd(nc, in_maps, core_ids=list(range(N_CORES)))
    LAST_RESULTS = res
    results = res.results

    cp_out = np.concatenate([r["cp_out"] for r in results], axis=1)[None]
    audio_out = np.concatenate([r["audio_out"] for r in results], axis=0)
    audio_out = audio_out.reshape(1, 1, FRAMES * WINDOW)
    return audio_out.astype(np.float32), cp_out.astype(np.float32)


# revision 6
# speedup vs baseline: 1.1817x; 1.1817x over previous
"""Trainium2 Bass kernel for nn_Block_65661460021475.

Computation (per reference, batch=1):
    x   = w1 @ relu(cp)                       # (128, F)
    y   = fft_convolve(x, dec_curve)          # == IIR: y[t] = dec*(y[t-1]+x[t])
    x2  = w2 @ y + x
    cp_out = tanh(g * x2)
    audio_out[w, f] = sum_c audio[w, c] cp_out[c, f]   -> stored (f, w)

Key reformulations:
  * The decay curve dec^(k+1) (dec in (0.75, 0.87)) underflows to zero in
    f32 after ~700 frames, so the length-131072 FFT convolution is exactly
    a first-order IIR filter.  It runs natively on the VectorEngine via
    tensor_tensor_scan: state = (x[t] + state) * dec.
  * Frames are sharded over 8 cores with a 512-frame halo warm-up for the
    scan (truncation error ~dec^512 ~ 1e-33, far below f32 eps).
  * The residual "+ x" is folded algebraically: x[t] = y[t]/dec - y[t-1],
    so  x2 = (w2 + diag(1/dec)) @ y - shift(y).  The shift term is applied
    as a -I matmul accumulated into the same PSUM bank, which avoids ever
    materializing x in SBUF.
"""

from contextlib import ExitStack

import numpy as np

import concourse.bacc as bacc
import concourse.bass as bass  # noqa: F401  (AP helpers)
import concourse.mybir as mybir
import concourse.tile as tile
from concourse.bass_utils import run_bass_kernel_spmd

N_CORES = 8
D = 128              # channels (= partitions)
FRAMES = 65536
WINDOW = 512
SHARD = FRAMES // N_CORES      # 8192 frames per core
HALO = 512                     # IIR warm-up halo
CHUNK = SHARD + HALO           # 8704 frames processed per core
FT = 512                       # frame tile
NT = CHUNK // FT               # 17 tiles (tile 0 = halo, no outputs)

F32 = mybir.dt.float32

_CACHE = {}

# Results of the most recent device run (for test harness introspection).
LAST_RESULTS = None


def _emit(nc):
    cp_h = nc.dram_tensor("cp", [D, CHUNK], F32, kind="ExternalInput")
    w1t_h = nc.dram_tensor("w1t", [D, D], F32, kind="ExternalInput")
    w2pt_h = nc.dram_tensor("w2pt", [D, D], F32, kind="ExternalInput")
    nid_h = nc.dram_tensor("negident", [D, D], F32, kind="ExternalInput")
    audiot_h = nc.dram_tensor("audiot", [D, WINDOW], F32, kind="ExternalInput")
    decb_h = nc.dram_tensor("decb", [D, FT], F32, kind="ExternalInput")
    g_h = nc.dram_tensor("g", [D, 1], F32, kind="ExternalInput")

    cpout_h = nc.dram_tensor("cp_out", [D, SHARD], F32, kind="ExternalOutput")
    audio_h = nc.dram_tensor("audio_out", [SHARD, WINDOW], F32, kind="ExternalOutput")

    # DRAM view of audio_out ordered (tile, p, q, w): one 1 MiB DMA per
    # frame tile stores the SBUF (p=128, q=4, w=512) staging tile; frame
    # index = tile*512 + q*128 + p.
    audio_view = audio_h[:, :].rearrange("(t q p) w -> t p q w", q=4, p=D)

    CH = 2048  # load/store chunk (1 MiB DMAs), 4 frame tiles

    with ExitStack() as ctx:
        tc = ctx.enter_context(tile.TileContext(nc))
        consts = ctx.enter_context(tc.tile_pool(name="consts", bufs=1))
        cpin = ctx.enter_context(tc.tile_pool(name="cpin", bufs=2))
        relup = ctx.enter_context(tc.tile_pool(name="relup", bufs=2))
        ybig = ctx.enter_context(tc.tile_pool(name="ybig", bufs=1))
        cpoutp = ctx.enter_context(tc.tile_pool(name="cpoutp", bufs=2))
        aup = ctx.enter_context(tc.tile_pool(name="aup", bufs=3))
        psx = ctx.enter_context(tc.tile_pool(name="psx", bufs=2, space="PSUM"))
        psb = ctx.enter_context(tc.tile_pool(name="psb", bufs=2, space="PSUM"))
        psa = ctx.enter_context(tc.tile_pool(name="psa", bufs=2, space="PSUM"))

        w1t_sb = consts.tile([D, D], F32)
        w2pt_sb = consts.tile([D, D], F32)
        nid_sb = consts.tile([D, D], F32)
        audiot_sb = consts.tile([D, WINDOW], F32)
        decb_sb = consts.tile([D, FT], F32)
        g_sb = consts.tile([D, 1], F32)
        nc.sync.dma_start(out=w1t_sb, in_=w1t_h[:, :])
        nc.sync.dma_start(out=w2pt_sb, in_=w2pt_h[:, :])
        nc.sync.dma_start(out=nid_sb, in_=nid_h[:, :])
        nc.sync.dma_start(out=audiot_sb, in_=audiot_h[:, :])
        nc.sync.dma_start(out=decb_sb, in_=decb_h[:, :])
        nc.sync.dma_start(out=g_sb, in_=g_h[:, :])

        # y for the whole chunk lives in one contiguous SBUF buffer so the
        # scan carry and the shifted view are plain slices (34 KiB/partition).
        y_big = ybig.tile([D, CHUNK], F32)

        relu_big = None
        co_big = None
        for t in range(NT):
            f0 = t * FT
            if f0 % CH == 0:
                # load + relu a whole 2048-frame chunk (1 MiB DMA, one DVE op)
                csz = min(CH, CHUNK - f0)
                cp_c = cpin.tile([D, CH], F32, tag="cp_c")
                nc.sync.dma_start(out=cp_c[:, :csz], in_=cp_h[:, f0:f0 + csz])
                relu_big = relup.tile([D, CH], F32, tag="relu_big")
                nc.vector.tensor_scalar_max(relu_big[:, :csz], cp_c[:, :csz], 0.0)

            px = psx.tile([D, FT], F32, tag="px")
            nc.tensor.matmul(px, w1t_sb, relu_big[:, f0 % CH:f0 % CH + FT],
                             start=True, stop=True)

            y_t = y_big[:, f0:f0 + FT]
            init = 0.0 if t == 0 else y_big[:, f0 - 1:f0]
            nc.vector.tensor_tensor_scan(
                y_t, px, decb_sb, init,
                op0=mybir.AluOpType.add, op1=mybir.AluOpType.mult,
            )

            if t == 0:
                continue
            ot = t - 1  # output tile index
            oc = ot * FT % CH  # offset inside the cp_out staging chunk

            # x2 = (w2 + diag(1/dec)) @ y - shift(y)   (== w2 @ y + x)
            pb = psb.tile([D, FT], F32, tag="pb")
            nc.tensor.matmul(pb, w2pt_sb, y_t, start=True, stop=False)
            nc.tensor.matmul(pb, nid_sb, y_big[:, f0 - 1:f0 + FT - 1],
                             start=False, stop=True)

            if oc == 0:
                co_big = cpoutp.tile([D, CH], F32, tag="co_big")
            co_t = co_big[:, oc:oc + FT]
            nc.scalar.activation(
                co_t, pb, mybir.ActivationFunctionType.Tanh,
                scale=g_sb[:, 0:1],
            )

            au_sb = aup.tile([D, 4, WINDOW], F32, tag="au_sb")
            for h in range(2):  # two PSUM-bank-pair halves -> one copy each
                pa = psa.tile([D, 2, WINDOW], F32, tag="pa")
                for qq in range(2):
                    q = 2 * h + qq
                    nc.tensor.matmul(
                        pa[:, qq, :], co_t[:, q * 128:(q + 1) * 128], audiot_sb,
                        start=True, stop=True,
                    )
                if h == 0:
                    nc.vector.tensor_copy(au_sb[:, 0:2, :], pa)
                else:
                    nc.scalar.copy(au_sb[:, 2:4, :], pa)
            nc.sync.dma_start(out=audio_view[ot], in_=au_sb)

            if oc == CH - FT:  # staging chunk full -> 1 MiB cp_out store
                c0 = ot * FT - (CH - FT)
                nc.sync.dma_start(out=cpout_h[:, c0:c0 + CH], in_=co_big)

    return nc


def _get_nc():
    if "nc" not in _CACHE:
        nc = _emit(bacc.Bacc())
        nc.compile()  # bacc passes: split multi-waits, reg alloc, nop fusion
        _CACHE["nc"] = nc
    return _CACHE["nc"]


def _host_prep(cp, w1, w2, audio, decays, gains):
    cp = np.asarray(cp, dtype=np.float32)
    w1 = np.asarray(w1, dtype=np.float32)
    w2 = np.asarray(w2, dtype=np.float32)
    audio = np.asarray(audio, dtype=np.float32)
    decays64 = np.asarray(decays, dtype=np.float64)
    gains64 = np.asarray(gains, dtype=np.float64)

    dec64 = 1e-12 + 0.5 + (1.0 / (1.0 + np.exp(-decays64))) * 0.5
    dec = dec64.astype(np.float32)
    g = ((1.0 / (1.0 + np.exp(-gains64))) * 5.0).astype(np.float32)

    w1t = np.ascontiguousarray(w1.T)
    w2pt = np.ascontiguousarray(w2.T + np.diag(1.0 / dec64)).astype(np.float32)
    negident = (-np.eye(D)).astype(np.float32)
    audiot = np.ascontiguousarray(audio.T)
    decb = np.ascontiguousarray(np.broadcast_to(dec[:, None], (D, FT)))
    g2 = np.ascontiguousarray(g[:, None])

    cp_pad = np.concatenate([np.zeros((D, HALO), np.float32), cp[0]], axis=1)
    in_maps = []
    for k in range(N_CORES):
        sl = np.ascontiguousarray(cp_pad[:, k * SHARD:k * SHARD + CHUNK])
        in_maps.append({
            "cp": sl, "w1t": w1t, "w2pt": w2pt, "negident": negident,
            "audiot": audiot, "decb": decb, "g": g2,
        })
    return in_maps


def kernel(cp, w1, w2, audio, decays, gains):
    global LAST_RESULTS
    in_maps = _host_prep(cp, w1, w2, audio, decays, gains)
    nc = _get_nc()
    res = run_bass_kernel_spmd(nc, in_maps, core_ids=list(range(N_CORES)))
    LAST_RESULTS = res
    results = res.results

    cp_out = np.concatenate([r["cp_out"] for r in results], axis=1)[None]
    audio_out = np.concatenate([r["audio_out"] for r in results], axis=0)
    audio_out = audio_out.reshape(1, 1, FRAMES * WINDOW)
    return audio_out.astype(np.float32), cp_out.astype(np.float32)


# revision 9
# speedup vs baseline: 1.7021x; 1.4404x over previous
"""Trainium2 Bass kernel for nn_Block_65661460021475.

Computation (per reference, batch=1):
    x   = w1 @ relu(cp)                       # (128, F)
    y   = fft_convolve(x, dec_curve)          # == IIR: y[t] = dec*(y[t-1]+x[t])
    x2  = w2 @ y + x
    cp_out = tanh(g * x2)
    audio_out[w, f] = sum_c audio[w, c] cp_out[c, f]   -> stored (f, w)

Key reformulations:
  * The decay curve dec^(k+1) (dec in (0.75, 0.87)) underflows to zero in
    f32 after ~700 frames, so the length-131072 FFT convolution is exactly
    a first-order IIR filter.  It runs natively on the VectorEngine via
    tensor_tensor_scan: state = (x[t] + state) * dec.
  * Frames are sharded over 8 cores with a 512-frame halo warm-up for the
    scan (truncation error ~dec^512 ~ 1e-33, far below f32 eps).
  * The residual "+ x" is folded algebraically: x[t] = y[t]/dec - y[t-1],
    so  x2 = (w2 + diag(1/dec)) @ y - shift(y).  The shift term is applied
    as a -I matmul accumulated into the same PSUM bank, which avoids ever
    materializing x in SBUF.
"""

from contextlib import ExitStack

import numpy as np

import concourse.bacc as bacc
import concourse.bass as bass  # noqa: F401  (AP helpers)
import concourse.mybir as mybir
import concourse.tile as tile
from concourse.bass_utils import run_bass_kernel_spmd

N_CORES = 8
D = 128              # channels (= partitions)
FRAMES = 65536
WINDOW = 512
SHARD = FRAMES // N_CORES      # 8192 frames per core
HALO = 512                     # IIR warm-up halo
CHUNK = SHARD + HALO           # 8704 frames processed per core
FT = 512                       # frame tile
NT = CHUNK // FT               # 17 tiles (tile 0 = halo, no outputs)

F32 = mybir.dt.float32
F32R = mybir.dt.float32r

_CACHE = {}

# Results of the most recent device run (for test harness introspection).
LAST_RESULTS = None


def _emit(nc):
    cp_h = nc.dram_tensor("cp", [D, CHUNK], F32, kind="ExternalInput")
    w1t_h = nc.dram_tensor("w1t", [D, D], F32R, kind="ExternalInput")
    w2pt_h = nc.dram_tensor("w2pt", [D, D], F32R, kind="ExternalInput")
    nid_h = nc.dram_tensor("negident", [D, D], F32R, kind="ExternalInput")
    audiot_h = nc.dram_tensor("audiot", [D, WINDOW], F32R, kind="ExternalInput")
    decb_h = nc.dram_tensor("decb", [D, FT], F32, kind="ExternalInput")
    g_h = nc.dram_tensor("g", [D, 1], F32, kind="ExternalInput")

    cpout_h = nc.dram_tensor("cp_out", [D, SHARD], F32R, kind="ExternalOutput")
    audio_h = nc.dram_tensor("audio_out", [SHARD, WINDOW], F32, kind="ExternalOutput")

    # DRAM view of audio_out ordered (tile, p, q, w): one 1 MiB DMA per
    # frame tile stores the SBUF (p=128, q=4, w=512) staging tile; frame
    # index = tile*512 + q*128 + p.
    audio_view = audio_h[:, :].rearrange("(t q p) w -> t p q w", q=4, p=D)

    CH = 2048  # load/store chunk (1 MiB DMAs), 4 frame tiles
    # float32r streams 1 col/cycle through the PE (vs 4 for two-pass fp32)
    # at >=256 moving dim; every producer feeding a matmul must declare
    # float32r output (walrus checkMatmultFP32r).
    mm = nc.tensor.matmul

    with ExitStack() as ctx:
        tc = ctx.enter_context(tile.TileContext(nc))
        consts = ctx.enter_context(tc.tile_pool(name="consts", bufs=1))
        cpin = ctx.enter_context(tc.tile_pool(name="cpin", bufs=2))
        relup = ctx.enter_context(tc.tile_pool(name="relup", bufs=2))
        ybig = ctx.enter_context(tc.tile_pool(name="ybig", bufs=1))
        cpoutp = ctx.enter_context(tc.tile_pool(name="cpoutp", bufs=2))
        aup = ctx.enter_context(tc.tile_pool(name="aup", bufs=3))
        psx = ctx.enter_context(tc.tile_pool(name="psx", bufs=2, space="PSUM"))
        psb = ctx.enter_context(tc.tile_pool(name="psb", bufs=2, space="PSUM"))
        psa = ctx.enter_context(tc.tile_pool(name="psa", bufs=2, space="PSUM"))

        w1t_sb = consts.tile([D, D], F32R)
        w2pt_sb = consts.tile([D, D], F32R)
        nid_sb = consts.tile([D, D], F32R)
        audiot_sb = consts.tile([D, WINDOW], F32R)
        decb_sb = consts.tile([D, FT], F32)
        g_sb = consts.tile([D, 1], F32)
        nc.sync.dma_start(out=w1t_sb, in_=w1t_h[:, :])
        nc.sync.dma_start(out=w2pt_sb, in_=w2pt_h[:, :])
        nc.sync.dma_start(out=nid_sb, in_=nid_h[:, :])
        nc.sync.dma_start(out=audiot_sb, in_=audiot_h[:, :])
        nc.sync.dma_start(out=decb_sb, in_=decb_h[:, :])
        nc.sync.dma_start(out=g_sb, in_=g_h[:, :])

        # y for the whole chunk lives in one contiguous SBUF buffer so the
        # scan carry and the shifted view are plain slices (34 KiB/partition).
        y_big = ybig.tile([D, CHUNK], F32R)

        relu_big = None
        co_big = None
        for t in range(NT):
            f0 = t * FT
            if f0 % CH == 0:
                # load + relu a whole 2048-frame chunk (1 MiB DMA, one DVE op)
                csz = min(CH, CHUNK - f0)
                cp_c = cpin.tile([D, CH], F32, tag="cp_c")
                nc.sync.dma_start(out=cp_c[:, :csz], in_=cp_h[:, f0:f0 + csz])
                relu_big = relup.tile([D, CH], F32R, tag="relu_big")
                nc.vector.tensor_scalar_max(relu_big[:, :csz], cp_c[:, :csz], 0.0)

            px = psx.tile([D, FT], F32, tag="px")
            mm(px, w1t_sb, relu_big[:, f0 % CH:f0 % CH + FT],
               start=True, stop=True)

            y_t = y_big[:, f0:f0 + FT]
            init = 0.0 if t == 0 else y_big[:, f0 - 1:f0]
            nc.vector.tensor_tensor_scan(
                y_t, px, decb_sb, init,
                op0=mybir.AluOpType.add, op1=mybir.AluOpType.mult,
            )

            if t == 0:
                continue
            ot = t - 1  # output tile index
            oc = ot * FT % CH  # offset inside the cp_out staging chunk

            # x2 = (w2 + diag(1/dec)) @ y - shift(y)   (== w2 @ y + x)
            pb = psb.tile([D, FT], F32, tag="pb")
            mm(pb, w2pt_sb, y_t, start=True, stop=False)
            mm(pb, nid_sb, y_big[:, f0 - 1:f0 + FT - 1],
               start=False, stop=True)

            if oc == 0:
                co_big = cpoutp.tile([D, CH], F32R, tag="co_big")
            co_t = co_big[:, oc:oc + FT]
            nc.scalar.activation(
                co_t, pb, mybir.ActivationFunctionType.Tanh,
                scale=g_sb[:, 0:1],
            )

            au_sb = aup.tile([D, 4, WINDOW], F32, tag="au_sb")
            for h in range(2):  # two PSUM-bank-pair halves -> one copy each
                pa = psa.tile([D, 2, WINDOW], F32, tag="pa")
                for qq in range(2):
                    q = 2 * h + qq
                    mm(
                        pa[:, qq, :], co_t[:, q * 128:(q + 1) * 128], audiot_sb,
                        start=True, stop=True,
                    )
                if h == 0:
                    nc.vector.tensor_copy(au_sb[:, 0:2, :], pa)
                else:
                    nc.scalar.copy(au_sb[:, 2:4, :], pa)
            nc.sync.dma_start(out=audio_view[ot], in_=au_sb)

            if oc == CH - FT:  # staging chunk full -> 1 MiB cp_out store
                c0 = ot * FT - (CH - FT)
                nc.sync.dma_start(out=cpout_h[:, c0:c0 + CH], in_=co_big)

    return nc


def _get_nc():
    if "nc" not in _CACHE:
        nc = _emit(bacc.Bacc())
        nc.compile()  # bacc passes: split multi-waits, reg alloc, nop fusion
        _CACHE["nc"] = nc
    return _CACHE["nc"]


def _host_prep(cp, w1, w2, audio, decays, gains):
    cp = np.asarray(cp, dtype=np.float32)
    w1 = np.asarray(w1, dtype=np.float32)
    w2 = np.asarray(w2, dtype=np.float32)
    audio = np.asarray(audio, dtype=np.float32)
    decays64 = np.asarray(decays, dtype=np.float64)
    gains64 = np.asarray(gains, dtype=np.float64)

    dec64 = 1e-12 + 0.5 + (1.0 / (1.0 + np.exp(-decays64))) * 0.5
    dec = dec64.astype(np.float32)
    g = ((1.0 / (1.0 + np.exp(-gains64))) * 5.0).astype(np.float32)

    w1t = np.ascontiguousarray(w1.T)
    w2pt = np.ascontiguousarray(w2.T + np.diag(1.0 / dec64)).astype(np.float32)
    negident = (-np.eye(D)).astype(np.float32)
    audiot = np.ascontiguousarray(audio.T)
    decb = np.ascontiguousarray(np.broadcast_to(dec[:, None], (D, FT)))
    g2 = np.ascontiguousarray(g[:, None])

    cp_pad = np.concatenate([np.zeros((D, HALO), np.float32), cp[0]], axis=1)
    in_maps = []
    for k in range(N_CORES):
        sl = np.ascontiguousarray(cp_pad[:, k * SHARD:k * SHARD + CHUNK])
        in_maps.append({
            "cp": sl, "w1t": w1t, "w2pt": w2pt, "negident": negident,
            "audiot": audiot, "decb": decb, "g": g2,
        })
    return in_maps


def kernel(cp, w1, w2, audio, decays, gains):
    global LAST_RESULTS
    in_maps = _host_prep(cp, w1, w2, audio, decays, gains)
    nc = _get_nc()
    res = run_bass_kernel_spmd(nc, in_maps, core_ids=list(range(N_CORES)))
    LAST_RESULTS = res
    results = res.results

    cp_out = np.concatenate([r["cp_out"] for r in results], axis=1)[None]
    audio_out = np.concatenate([r["audio_out"] for r in results], axis=0)
    audio_out = audio_out.reshape(1, 1, FRAMES * WINDOW)
    return audio_out.astype(np.float32), cp_out.astype(np.float32)
